# revision 25
# baseline (speedup 1.0000x reference)
"""Trainium2 Bass kernel for the BottleneckIndependent MoE-routed conv block.

Math (per sample b):
  rw1 = sigmoid(mean_hw(x) @ r1_w + r1_b)                     [E]
  cw1 = sum_e rw1[e] * w1[e]          (per-sample 1x1 weights)
  out1 = relu(bn1(cw1 @ x))
  rw2 / cw2 / out2: same with 3x3 conv (pad 1)
  rw3 / cw3: 1x1; out = relu(bn3(cw3 @ out2) + x)

Strategy (8 cores, data-parallel over batch, 4 samples/core):
  * BN scales fold into expert weights on the host; BN bias + ReLU fuse into
    one ScalarE/VectorE epilogue op per output chunk.
  * The rank-8 expert combine runs on the PE with the expert weights as the
    STATIONARY operand ([128,128] chunks, rows=(j,e), j an o-subgroup index)
    against a block-diagonal routing matrix bd[128, 64] (cols=(b,j)).  This
    yields combined weights directly in [i_partition, (b,o)] layout -- the
    exact lhsT layout the conv matmuls need.  bd is built without
    cross-partition ops: routing weights are host-replicated (col m = expert
    m%8) so the routing matmul emits a partition-replicated pre-sigmoid,
    then one masked multiply forms bd.
  * Convs are per-sample matmuls (contraction = input channels); the 3x3 is
    9 shifted 1x1 matmuls accumulating in PSUM over a zero-padded 16x16
    buffer.  The residual add is an identity matmul into the conv3 group.
  * Weight chunk order in DRAM matches consumption order (output-channel
    halves first), so convs pipeline directly behind the weight DMA.
  * DMA discipline: the per-core HBM port is ~293 GB/s and each dma_start
    costs ~0.65us of dispatch on its engine stream, so transfers are few and
    large; x+w1 dispatch on SP, w2/w3 on the otherwise-idle GpSimd SWDGE
    chained behind w1 so early phases get the full port.  Output is bf16
    (host upcasts) to halve writeback.
"""

import numpy as np
import ml_dtypes

B, INP, WIDTH, OUTP, E, H = 32, 1024, 256, 1024, 8, 14
EPS = 1e-5
S = H * H            # 196
SP = 256             # 16*16 padded spatial
NCORES = 8
BS = B // NCORES     # 4 samples per core
P = 128

BF16 = ml_dtypes.bfloat16

N_CHUNKS1 = 8 * 16            # stage1: (g, ic)   c1 = g*8 + ic
N_CHUNKS2 = 9 * 2 * 16        # stage2: (gh, tap, ic, gl) c2 = gh*144+tap*16+ic*8+gl
N_CHUNKS3 = 2 * 64            # stage3: (g, ic)   c3 = g*2 + ic

_nc_cache = None
last_exec_time_ns = None
last_trace_path = None
last_res = None


# ----------------------------------------------------------------------------
# Host-side input preparation (pure numpy)
# ----------------------------------------------------------------------------

def _fold_bn(g, b, m, v):
    inv = (g / np.sqrt(v + EPS)).astype(np.float32)
    beta = (b - m * inv).astype(np.float32)
    return inv, beta


def _prep_weights(w1, w2, w3, r1_w, r1_b, r2_w, r2_b, r3_w, r3_b,
                  bn1_g, bn1_b, bn1_m, bn1_v, bn2_g, bn2_b, bn2_m, bn2_v,
                  bn3_g, bn3_b, bn3_m, bn3_v):
    inv1, beta1 = _fold_bn(bn1_g, bn1_b, bn1_m, bn1_v)
    inv2, beta2 = _fold_bn(bn2_g, bn2_b, bn2_m, bn2_v)
    inv3, beta3 = _fold_bn(bn3_g, bn3_b, bn3_m, bn3_v)

    w1p = (w1[:, :, :, 0, 0] * inv1[None, :, None]).astype(np.float32)  # [E,256,1024]
    w2p = (w2 * inv2[None, :, None, None, None]).astype(np.float32)     # [E,256,256,3,3]
    w3p = (w3[:, :, :, 0, 0] * inv3[None, :, None]).astype(np.float32)  # [E,1024,256]

    # stage1 rows (j,e); chunk c1 = g*8+ic; value = w1p[e, g*16+j, ic*128+ip]
    a = w1p.reshape(E, 16, 16, 8, P)                    # e, g, j, ic, ip
    w1r = a.transpose(2, 0, 1, 3, 4).reshape(P, N_CHUNKS1 * P).astype(BF16)

    # stage2 chunk c2 = gh*144 + tap*16 + ic*8 + gl; g = gh*8+gl
    # value = w2p[e, g*16+j, ic*128+ip, kh, kw]
    a = w2p.reshape(E, 2, 8, 16, 2, P, 3, 3)            # e, gh, gl, j, ic, ip, kh, kw
    w2r = a.transpose(3, 0, 1, 6, 7, 4, 2, 5).reshape(P, N_CHUNKS2 * P).astype(BF16)

    # stage3 chunk c3 = g*2+ic; value = w3p[e, g*16+j, ic*128+ip]
    a = w3p.reshape(E, 64, 16, 2, P)                    # e, g, j, ic, ip
    w3r = a.transpose(2, 0, 1, 3, 4).reshape(P, N_CHUNKS3 * P).astype(BF16)

    def rep_routing(rw, nchunks):
        # [C, E] -> [128, nchunks*128]; col m of chunk ic = rw[ic*128+p, m%8]/S
        r = (np.asarray(rw, np.float32) / float(S)).reshape(nchunks, P, E)
        rrep = np.tile(r[:, :, None, :], (1, 1, 16, 1)).reshape(nchunks, P, P)
        return rrep.transpose(1, 0, 2).reshape(P, nchunks * P)

    # bf16 constant pack: mask(64) | ident(128) | r1rep(1024) | r2rep(256) | r3rep(256)
    jj = np.arange(P)[:, None] // 8
    col_j = np.tile(np.arange(16), 4)[None, :]
    mask = (col_j == jj).astype(np.float32)
    ident = np.eye(P, dtype=np.float32)
    cbf = np.concatenate(
        [mask, ident, rep_routing(r1_w, 8), rep_routing(r2_w, 2),
         rep_routing(r3_w, 2)], axis=1).astype(BF16)    # [128, 1728]

    # f32 constant pack: rb(3) | beta1(2) | beta2(2) | beta3(8)
    rb = np.stack([np.tile(np.asarray(r, np.float32), 16)
                   for r in (r1_b, r2_b, r3_b)], axis=1)
    cf32 = np.concatenate(
        [rb, beta1.reshape(2, P).T, beta2.reshape(2, P).T,
         beta3.reshape(8, P).T], axis=1).astype(np.float32)  # [128, 15]

    return dict(w1r=w1r, w2r=w2r, w3r=w3r, cbf=cbf, cf32=cf32)


def _prep_x(x):
    out = []
    for c in range(NCORES):
        xc = np.asarray(x[c * BS:(c + 1) * BS], np.float32)
        xb = xc.reshape(BS, 8, P, S).transpose(2, 0, 1, 3).reshape(P, BS * 8 * S)
        out.append(np.ascontiguousarray(xb.astype(BF16)))
    return out


# ----------------------------------------------------------------------------
# Device program
# ----------------------------------------------------------------------------

def _build_nc():
    import concourse.tile as tile
    import concourse.mybir as mybir
    from concourse.bacc import Bacc
    from concourse.tile_rust import add_dep_helper
    from contextlib import ExitStack

    f32 = mybir.dt.float32
    bf16 = mybir.dt.bfloat16
    AF = mybir.ActivationFunctionType
    ALU = mybir.AluOpType
    AX = mybir.AxisListType

    nc = Bacc("TRN2")

    xd = nc.dram_tensor("x_bf", [P, BS * 8 * S], bf16, kind="ExternalInput")
    w1d = nc.dram_tensor("w1r", [P, N_CHUNKS1 * P], bf16, kind="ExternalInput")
    w2d = nc.dram_tensor("w2r", [P, N_CHUNKS2 * P], bf16, kind="ExternalInput")
    w3d = nc.dram_tensor("w3r", [P, N_CHUNKS3 * P], bf16, kind="ExternalInput")
    cbfd = nc.dram_tensor("cbf", [P, 1728], bf16, kind="ExternalInput")
    cf32d = nc.dram_tensor("cf32", [P, 15], f32, kind="ExternalInput")
    outd = nc.dram_tensor("out", [P, BS * 8 * S], bf16, kind="ExternalOutput")

    with tile.TileContext(nc) as tc, ExitStack() as ctx:
        singles = ctx.enter_context(tc.tile_pool(name="singles", bufs=1))
        wbig = ctx.enter_context(tc.tile_pool(name="wbig", bufs=1))
        cwa = ctx.enter_context(tc.tile_pool(name="cwa", bufs=1))
        ostage = ctx.enter_context(tc.tile_pool(name="ostage", bufs=4))
        kpsum = ctx.enter_context(tc.tile_pool(name="kpsum", bufs=4, space="PSUM"))
        cpsum = ctx.enter_context(tc.tile_pool(name="cpsum", bufs=3, space="PSUM"))
        rpsum = ctx.enter_context(tc.tile_pool(name="rpsum", bufs=1, space="PSUM"))

        # ---- constants + x + w1 on the SP HWDGE queue --------------------
        cbf_sb = singles.tile([P, 1728], bf16)
        nc.sync.dma_start(out=cbf_sb, in_=cbfd[:, :])
        cf32_sb = singles.tile([P, 15], f32)
        nc.sync.dma_start(out=cf32_sb, in_=cf32d[:, :])
        mask_sb = cbf_sb[:, 0:64]
        ident_sb = cbf_sb[:, 64:192]
        r1w_sb = cbf_sb[:, 192:1216]
        r2w_sb = cbf_sb[:, 1216:1472]
        r3w_sb = cbf_sb[:, 1472:1728]
        rb_sb = cf32_sb[:, 0:3]
        beta_sb = cf32_sb[:, 3:15]

        x_sb = singles.tile([P, BS * 8 * S], bf16)
        for b in range(BS):
            nc.sync.dma_start(out=x_sb[:, b * 8 * S:(b + 1) * 8 * S],
                              in_=xd[:, b * 8 * S:(b + 1) * 8 * S])

        w1_sb = wbig.tile([P, N_CHUNKS1 * P], bf16, tag="wbig", name="w1_sb")
        w1_last = None
        for sl in range(4):
            w = N_CHUNKS1 * P // 4
            w1_last = nc.sync.dma_start(out=w1_sb[:, sl * w:(sl + 1) * w],
                                        in_=w1d[:, sl * w:(sl + 1) * w])

        # ---- w2 on GpSimd SWDGE, strictly after w1 -----------------------
        w2_sb = singles.tile([P, N_CHUNKS2 * P], bf16)
        w2_dmas = []
        for sl in range(6):
            w = N_CHUNKS2 * P // 6
            d = nc.gpsimd.dma_start(out=w2_sb[:, sl * w:(sl + 1) * w],
                                    in_=w2d[:, sl * w:(sl + 1) * w])
            add_dep_helper(d.ins, w1_last.ins, sync=True,
                           reason="w2 after w1 (hbm port order)")
            if sl >= 2:
                add_dep_helper(d.ins, w2_dmas[sl - 2].ins, sync=True,
                               reason="w2 slice order (depth-2 pipeline)")
            w2_dmas.append(d)
        w2_last = w2_dmas[-1]

        cw1 = cwa.tile([P, BS * 8 * 256], bf16, tag="cwa", name="cw1")
        cw2 = singles.tile([P, BS * 9 * 2 * 256], bf16)
        out1pad = singles.tile([P, BS * 2 * SP], bf16)
        nc.vector.memset(out1pad, 0.0)
        out2 = singles.tile([P, BS * 2 * S], bf16)

        pool1 = singles.tile([P, 8 * BS], f32)
        pool1b = singles.tile([P, 8 * BS], bf16)
        pool2 = singles.tile([P, 2 * BS], f32)
        pool2b = singles.tile([P, 2 * BS], bf16)
        pool3 = singles.tile([P, 2 * BS], f32)
        pool3b = singles.tile([P, 2 * BS], bf16)

        x_v = x_sb.rearrange("p (b c s) -> p b c s", b=BS, c=8)
        mask_v = mask_sb.rearrange("p (b j) -> p b j", b=BS)
        out1pad_v = out1pad.rearrange("p (b c h w) -> p b c h w", b=BS, c=2, h=16)

        # ---- pooling 1 (mean over spatial; 1/S folded into routing w) ----
        pool1_v = pool1.rearrange("p (c b) -> p c b", b=BS)
        for b in range(BS):
            nc.vector.tensor_reduce(out=pool1_v[:, :, b], in_=x_v[:, b],
                                    axis=AX.X, op=ALU.add)
        nc.vector.tensor_copy(out=pool1b, in_=pool1)

        # ---- routing helper ----------------------------------------------
        def routing(st, rw_sb, pool_bf, nchunks):
            ps = rpsum.tile([P, BS], f32, tag="rps", name=f"ps_rt{st}")
            for ic in range(nchunks):
                nc.tensor.matmul(ps, rw_sb[:, ic * P:(ic + 1) * P],
                                 pool_bf[:, ic * BS:(ic + 1) * BS],
                                 start=(ic == 0), stop=(ic == nchunks - 1))
            rwt = singles.tile([P, BS], bf16, name=f"rwt{st}")
            nc.scalar.activation(out=rwt, in_=ps, func=AF.Sigmoid,
                                 bias=rb_sb[:, st:st + 1], scale=1.0)
            bd = singles.tile([P, BS * 16], bf16, name=f"bd{st}")
            bd_v = bd.rearrange("p (b j) -> p b j", b=BS)
            nc.vector.tensor_tensor(
                out=bd_v, in0=mask_v,
                in1=rwt[:, :, None].to_broadcast((P, BS, 16)),
                op=ALU.mult)
            return bd

        # ---- combine helper: one bank = 8 chunks -> 1-2 copies -----------
        def combine(st, w_sb, bd, nbanks, copy_fn):
            for bank in range(nbanks):
                ps = kpsum.tile([P, 512], f32, tag="kps", name=f"ps_cmb{st}")
                for c8 in range(8):
                    c = bank * 8 + c8
                    nc.tensor.matmul(ps[:, c8 * 64:(c8 + 1) * 64],
                                     w_sb[:, c * P:(c + 1) * P], bd,
                                     start=True, stop=True)
                copy_fn(bank, ps)

        # ================== stage 1 =======================================
        bd1 = routing(0, r1w_sb, pool1b, 8)
        # chunk c1 = g*8+ic; bank i = g=i (8 ic chunks)
        cw1_v = cw1.rearrange("p (b ic g j) -> p g ic b j", b=BS, ic=8, g=16, j=16)

        def copy1(bank, ps):
            src = ps.rearrange("p (ic b j) -> p ic b j", ic=8, b=BS)
            if bank % 2 == 0:
                nc.vector.tensor_copy(out=cw1_v[:, bank], in_=src)
            else:
                nc.scalar.copy(cw1_v[:, bank], src)
        combine(1, w1_sb, bd1, N_CHUNKS1 // 8, copy1)

        # w3 DMA (GpSimd) after combine1 frees the wbig slot, after w2 for bw
        w3_sb = wbig.tile([P, N_CHUNKS3 * P], bf16, tag="wbig", name="w3_sb")
        w3_dmas = []
        for sl in range(4):
            w = N_CHUNKS3 * P // 4
            d = nc.gpsimd.dma_start(out=w3_sb[:, sl * w:(sl + 1) * w],
                                    in_=w3d[:, sl * w:(sl + 1) * w])
            add_dep_helper(d.ins, w2_last.ins, sync=True,
                           reason="w3 after w2 (hbm port order)")
            if sl >= 2:
                add_dep_helper(d.ins, w3_dmas[sl - 2].ins, sync=True,
                               reason="w3 slice order (depth-2 pipeline)")
            w3_dmas.append(d)

        # conv1 + bn1 + relu (oc-major to follow weight arrival)
        for oc in range(2):
            for b in range(BS):
                ps = cpsum.tile([P, S], f32, tag="cps", name="ps_c1")
                for ic in range(8):
                    nc.tensor.matmul(
                        ps, cw1[:, b * 2048 + ic * 256 + oc * P:
                                b * 2048 + ic * 256 + oc * P + P],
                        x_sb[:, b * 8 * S + ic * S:b * 8 * S + (ic + 1) * S],
                        start=(ic == 0), stop=(ic == 7))
                if (b * 2 + oc) % 2 == 0:
                    nc.scalar.activation(
                        out=out1pad_v[:, b, oc, 1:15, 1:15],
                        in_=ps.rearrange("p (h w) -> p h w", h=H),
                        func=AF.Relu, bias=beta_sb[:, oc:oc + 1], scale=1.0)
                else:
                    nc.vector.tensor_scalar(
                        out=out1pad_v[:, b, oc, 1:15, 1:15],
                        in0=ps.rearrange("p (h w) -> p h w", h=H),
                        scalar1=beta_sb[:, oc:oc + 1], scalar2=0.0,
                        op0=ALU.add, op1=ALU.max)

        # pool2
        for b in range(BS):
            for oc in range(2):
                nc.vector.tensor_reduce(
                    out=pool2[:, oc * BS + b:oc * BS + b + 1],
                    in_=out1pad[:, (b * 2 + oc) * SP:(b * 2 + oc + 1) * SP],
                    axis=AX.X, op=ALU.add)
        nc.vector.tensor_copy(out=pool2b, in_=pool2)

        # ================== stage 2 =======================================
        bd2 = routing(1, r2w_sb, pool2b, 2)
        # chunk c2 = gh*144 + tap*16 + ic*8 + gl; bank = gh*18 + tap*2 + ic
        cw2_v = cw2.rearrange("p (b t ic gh gl j) -> p gh t ic gl b j",
                              b=BS, t=9, ic=2, gh=2, gl=8, j=16)

        def copy2(bank, ps):
            gh, tap, ic = bank // 18, (bank % 18) // 2, bank % 2
            src = ps.rearrange("p (gl b j) -> p gl b j", gl=8, b=BS)
            dst = cw2_v[:, gh, tap, ic]
            if bank % 2 == 0:
                nc.vector.tensor_copy(out=dst, in_=src)
            else:
                nc.scalar.copy(dst, src)
        combine(2, w2_sb, bd2, N_CHUNKS2 // 8, copy2)

        # conv2 + bn2 + relu (oc-major: oc==gh half)
        for oc in range(2):
            for b in range(BS):
                ps = cpsum.tile([P, S], f32, tag="cps", name="ps_c2")
                k = 0
                for ic in range(2):
                    for tap in range(9):
                        kh, kw = tap // 3, tap % 3
                        nc.tensor.matmul(
                            ps.rearrange("p (h w) -> p h w", h=H),
                            cw2[:, b * 4608 + tap * 512 + ic * 256 + oc * P:
                                b * 4608 + tap * 512 + ic * 256 + oc * P + P],
                            out1pad_v[:, b, ic, kh:kh + H, kw:kw + H],
                            start=(k == 0), stop=(k == 17))
                        k += 1
                if (b * 2 + oc) % 2 == 0:
                    nc.scalar.activation(
                        out=out2[:, (b * 2 + oc) * S:(b * 2 + oc + 1) * S],
                        in_=ps, func=AF.Relu, bias=beta_sb[:, 2 + oc:3 + oc],
                        scale=1.0)
                else:
                    nc.vector.tensor_scalar(
                        out=out2[:, (b * 2 + oc) * S:(b * 2 + oc + 1) * S],
                        in0=ps, scalar1=beta_sb[:, 2 + oc:3 + oc], scalar2=0.0,
                        op0=ALU.add, op1=ALU.max)

        # pool3
        for b in range(BS):
            for oc in range(2):
                nc.vector.tensor_reduce(
                    out=pool3[:, oc * BS + b:oc * BS + b + 1],
                    in_=out2[:, (b * 2 + oc) * S:(b * 2 + oc + 1) * S],
                    axis=AX.X, op=ALU.add)
        nc.vector.tensor_copy(out=pool3b, in_=pool3)

        # ================== stage 3 =======================================
        bd3 = routing(2, r3w_sb, pool3b, 2)
        cw3 = cwa.tile([P, BS * 2 * 1024], bf16, tag="cwa", name="cw3")
        # chunk c3 = g*2+ic; bank i covers g in [4i, 4i+4), both ic
        cw3_v = cw3.rearrange("p (b ic g j) -> p ic g b j", b=BS, ic=2, g=64, j=16)

        def copy3(bank, ps):
            src = ps.rearrange("p (g ic b j) -> p g ic b j", g=4, ic=2, b=BS)
            for ic in range(2):
                dst = cw3_v[:, ic, bank * 4:bank * 4 + 4]
                if (bank + ic) % 2 == 0:
                    nc.vector.tensor_copy(out=dst, in_=src[:, :, ic])
                else:
                    nc.scalar.copy(dst, src[:, :, ic])
        combine(3, w3_sb, bd3, N_CHUNKS3 // 8, copy3)

        # conv3 + identity + bn3-bias + relu -> out (oc-major, bf16 out)
        outd_v = outd.rearrange("p (b c s) -> p b c s", b=BS, c=8)
        for oc in range(8):
            ost = ostage.tile([P, BS, S], bf16, tag="ost", name="ost")
            for b in range(BS):
                ps = cpsum.tile([P, S], f32, tag="cps", name="ps_c3")
                nc.tensor.matmul(
                    ps, cw3[:, b * 2048 + 0 * 1024 + oc * P:
                            b * 2048 + 0 * 1024 + oc * P + P],
                    out2[:, b * 2 * S:b * 2 * S + S], start=True, stop=False)
                nc.tensor.matmul(
                    ps, cw3[:, b * 2048 + 1 * 1024 + oc * P:
                            b * 2048 + 1 * 1024 + oc * P + P],
                    out2[:, b * 2 * S + S:b * 2 * S + 2 * S],
                    start=False, stop=False)
                nc.tensor.matmul(
                    ps, ident_sb,
                    x_sb[:, b * 8 * S + oc * S:b * 8 * S + (oc + 1) * S],
                    start=False, stop=True)
                if (oc + b) % 2 == 0:
                    nc.scalar.activation(
                        out=ost[:, b], in_=ps, func=AF.Relu,
                        bias=beta_sb[:, 4 + oc:5 + oc], scale=1.0)
                else:
                    nc.vector.tensor_scalar(
                        out=ost[:, b], in0=ps, scalar1=beta_sb[:, 4 + oc:5 + oc],
                        scalar2=0.0, op0=ALU.add, op1=ALU.max)
            nc.sync.dma_start(out=outd_v[:, :, oc], in_=ost)

    nc.finalize()
    return nc


# ----------------------------------------------------------------------------
# Entry point
# ----------------------------------------------------------------------------

def kernel(x, w1, w2, w3, r1_w, r1_b, r2_w, r2_b, r3_w, r3_b,
           bn1_g, bn1_b, bn1_m, bn1_v, bn2_g, bn2_b, bn2_m, bn2_v,
           bn3_g, bn3_b, bn3_m, bn3_v, _trace=False):
    global _nc_cache, last_exec_time_ns, last_trace_path, last_res
    from concourse.bass_utils import run_bass_kernel_spmd

    prep = _prep_weights(
        np.asarray(w1, np.float32), np.asarray(w2, np.float32),
        np.asarray(w3, np.float32),
        np.asarray(r1_w, np.float32), np.asarray(r1_b, np.float32),
        np.asarray(r2_w, np.float32), np.asarray(r2_b, np.float32),
        np.asarray(r3_w, np.float32), np.asarray(r3_b, np.float32),
        np.asarray(bn1_g, np.float32), np.asarray(bn1_b, np.float32),
        np.asarray(bn1_m, np.float32), np.asarray(bn1_v, np.float32),
        np.asarray(bn2_g, np.float32), np.asarray(bn2_b, np.float32),
        np.asarray(bn2_m, np.float32), np.asarray(bn2_v, np.float32),
        np.asarray(bn3_g, np.float32), np.asarray(bn3_b, np.float32),
        np.asarray(bn3_m, np.float32), np.asarray(bn3_v, np.float32))
    xs = _prep_x(np.asarray(x, np.float32))

    shared_map = {
        "w1r": prep["w1r"], "w2r": prep["w2r"], "w3r": prep["w3r"],
        "cbf": prep["cbf"], "cf32": prep["cf32"],
    }
    in_maps = [dict(shared_map, x_bf=xs[c]) for c in range(NCORES)]

    if _nc_cache is None:
        _nc_cache = _build_nc()
    res = run_bass_kernel_spmd(_nc_cache, in_maps, core_ids=list(range(NCORES)),
                               trace=_trace)
    last_exec_time_ns = res.exec_time_ns
    last_trace_path = (res.instructions_and_trace or (None, None))[1]
    last_res = res

    out = np.empty((B, OUTP, H, H), np.float32)
    for c in range(NCORES):
        o = np.asarray(res.results[c]["out"], np.float32)   # [128, BS*8*196]
        out[c * BS:(c + 1) * BS] = (
            o.reshape(P, BS, 8, S).transpose(1, 2, 0, 3).reshape(BS, OUTP, H, H))
    return out
